# revision 10
# baseline (speedup 1.0000x reference)
"""Trainium2 Bass kernel for ContextAttention (Bahdanau-style additive attention).

Math (per example b):
  q = W_in @ x + b_in                                  [H]
  ctx = W_ctx @ contexts[b].T + b_ctx                  [H, S]
  att = V . tanh(q[:,None] + ctx)                      [S]
  att = where(mask, -inf, att); score = softmax(att)   [S]
  hidden = ctx @ score = W_ctx @ (contexts[b].T @ score) + b_ctx   (sum(score)==1)

Sharding: pure data parallel, batch 128 -> 8 cores x 16 examples.
Weights replicated. All heavy compute on-chip in bf16 with f32 accumulation.
"""

from contextlib import ExitStack

import ml_dtypes
import numpy as np

import concourse.bass as bass
import concourse.tile as tile
from concourse import bacc, mybir
from concourse.bass_utils import run_bass_kernel_spmd

B, S, D, H = 128, 2048, 512, 512
NCORES = 8
BL = B // NCORES        # 16 examples per core
KD = D // 128           # 4 contraction chunks
MH = H // 128           # 4 h tiles
NS = S // 512           # 4 s chunks of 512
STS = S // 128          # 16 s tiles of 128
SBLK = 4                # s-subtiles per load block (512 rows per cast-DMA)
GRP = 4                 # softmax group size (examples)

F32 = mybir.dt.float32
BF16 = mybir.dt.bfloat16
AF = mybir.ActivationFunctionType
ALU = mybir.AluOpType
BF16_NP = ml_dtypes.bfloat16


def _build_kernel(ctx: ExitStack, tc: tile.TileContext, outs, ins):
    nc = tc.nc
    hiddenT_o, score_o = outs
    contexts_i, inputsT_i, maskadd_i, W_inT_i, W_ctxT_i, V_i, biases_i = ins

    const = ctx.enter_context(tc.tile_pool(name="const", bufs=1))
    cnat_p = ctx.enter_context(tc.tile_pool(name="cnat", bufs=18))
    ctxT_p = ctx.enter_context(tc.tile_pool(name="ctxT", bufs=8))
    tanh_p = ctx.enter_context(tc.tile_pool(name="tanh", bufs=4))
    row_p = ctx.enter_context(tc.tile_pool(name="row", bufs=4))
    sct_p = ctx.enter_context(tc.tile_pool(name="sct", bufs=32))
    ps_proj = ctx.enter_context(tc.tile_pool(name="psp", bufs=2, space="PSUM"))
    ps_att = ctx.enter_context(tc.tile_pool(name="psa", bufs=1, space="PSUM"))
    ps_w = ctx.enter_context(tc.tile_pool(name="psw", bufs=1, space="PSUM"))
    ps_sm = ctx.enter_context(tc.tile_pool(name="pss", bufs=1, space="PSUM"))
    dram_p = ctx.enter_context(tc.tile_pool(name="dscr", bufs=3, space="DRAM"))

    # ---- constants ----
    Wc = const.tile([128, KD * H], BF16, tag="Wc")      # [d128, (k,h)] W_ctx.T
    Wi = const.tile([128, KD * H], BF16, tag="Wi")       # [d128, (k,h)] W_in.T
    inT = const.tile([128, KD * BL], BF16, tag="inT")    # [d128, (k,b)]
    Vt = const.tile([128, MH], BF16, tag="Vt")          # [h128, m]
    bia = const.tile([128, 2 * MH], F32, tag="bia")     # b_in chunks 0..3, b_ctx 4..7
    for k in range(KD):
        nc.sync.dma_start(Wc[:, k * H:(k + 1) * H], W_ctxT_i[k * 128:(k + 1) * 128, :])
        nc.sync.dma_start(Wi[:, k * H:(k + 1) * H], W_inT_i[k * 128:(k + 1) * 128, :])
        nc.sync.dma_start(inT[:, k * BL:(k + 1) * BL], inputsT_i[k * 128:(k + 1) * 128, :])
    nc.sync.dma_start(Vt[:], V_i[:, :])
    nc.sync.dma_start(bia[:], biases_i[:, :])

    # ---- q = W_in @ x + b_in + b_ctx  -> Q [h128, (m,b)] f32 ----
    Q = const.tile([128, MH * BL], F32, tag="Q")
    for m in range(MH):
        qp = ps_sm.tile([128, BL], F32, tag="qps")
        for k in range(KD):
            nc.tensor.matmul(qp[:], Wi[:, k * H + m * 128: k * H + (m + 1) * 128],
                             inT[:, k * BL:(k + 1) * BL],
                             start=(k == 0), stop=(k == KD - 1))
        nc.vector.tensor_scalar_add(Q[:, m * BL:(m + 1) * BL], qp[:],
                                    bia[:, m:m + 1])

    # ---- softmax group pool (partition-0-based tiles) ----
    smx_p = ctx.enter_context(tc.tile_pool(name="smx", bufs=2))
    wct_b = const.tile([BL, D], BF16, tag="wct_b")
    att_grp = None

    cnat = {}   # (b, blk) -> tile [128, SBLK*D] bf16, free=(t, d)
    for b in range(BL):
        if b % GRP == 0:
            att_grp = smx_p.tile([GRP, S], F32, tag="att_grp")
            mad_g = smx_p.tile([GRP, S], F32, tag="mad_g")
            nc.sync.dma_start(mad_g[:], maskadd_i[b:b + GRP, :])
        # ---- load + cast f32->bf16 (natural layout), 512 s-rows per DMA;
        # bounce bf16 through DRAM scratch so the transpose can be done with
        # 4 wide xbar instructions (in_=[2048,128] DRAM) instead of 64 small
        # SBUF ones (fixed ~0.8us sequencer cost per transpose instruction).
        scr = dram_p.tile([S, D], BF16, tag="scr", name=f"scr{b}")
        for blk in range(STS // SBLK):
            cn = cnat_p.tile([128, SBLK * D], BF16, tag="cn")
            src = contexts_i[b, blk * SBLK * 128:(blk + 1) * SBLK * 128, :]
            nc.gpsimd.dma_start(
                cn[:].rearrange("p (t d) -> p t d", t=SBLK),
                src.rearrange("(t p) d -> p t d", p=128))
            nc.sync.dma_start(
                scr[blk * SBLK * 128:(blk + 1) * SBLK * 128, :]
                .rearrange("(t p) d -> p t d", p=128),
                cn[:].rearrange("p (t d) -> p t d", t=SBLK))
            cnat[(b, blk)] = cn

        def cn_sub(b_, st):
            t = cnat[(b_, st // SBLK)]
            return t[:, (st % SBLK) * D:(st % SBLK) * D + D]

        # ---- wide xbar transpose from DRAM -> ctxT[k] [d128, S] bf16 ----
        ctxT = [ctxT_p.tile([128, S], BF16, tag="ct", name=f"ct{b}_{k}")
                for k in range(KD)]
        for k in range(KD):
            nc.sync.dma_start(ctxT[k][:], scr[:, k * 128:(k + 1) * 128],
                              transpose=True)

        # ---- projection + tanh + V-reduce ----
        att_row = row_p.tile([1, S], F32, tag="att_row", bufs=2)
        abig = ps_att.tile([1, S], F32, tag="attps")
        for n in range(NS):
            ap = abig[:, n * 512:(n + 1) * 512]
            for m in range(MH):
                pp = ps_proj.tile([128, 512], F32, tag="proj")
                for k in range(KD):
                    nc.tensor.matmul(
                        pp[:], Wc[:, k * H + m * 128: k * H + (m + 1) * 128],
                        ctxT[k][:, n * 512:(n + 1) * 512],
                        start=(k == 0), stop=(k == KD - 1))
                th = tanh_p.tile([128, 512], BF16, tag="th")
                nc.scalar.activation(th[:], pp[:], AF.Tanh,
                                     bias=Q[:, m * BL + b: m * BL + b + 1])
                nc.tensor.matmul(ap, Vt[:, m:m + 1], th[:],
                                 start=(m == 0), stop=(m == MH - 1),
                                 skip_group_check=True)
        nc.vector.tensor_copy(att_row[:], abig[:])
        bg = b % GRP
        nc.sync.dma_start(att_grp[bg:bg + 1, :], att_row[:])

        # ---- per-group: softmax + scoresT + weighted context sum ----
        if b % GRP == GRP - 1:
            g0 = b - (GRP - 1)
            sco_f = smx_p.tile([GRP, S], F32, tag="sco_f")
            sco_b = smx_p.tile([BL, S], BF16, tag="sco_b")  # rows GRP.. unused pad
            nmax = smx_p.tile([GRP, 1], F32, tag="nmax")
            sume = smx_p.tile([GRP, 1], F32, tag="sume")
            rsum = smx_p.tile([GRP, 1], F32, tag="rsum")
            nc.vector.tensor_tensor(att_grp[:], att_grp[:], mad_g[:], ALU.add)
            nc.vector.tensor_reduce(nmax[:], att_grp[:],
                                    axis=mybir.AxisListType.X, op=ALU.max,
                                    negate=True)
            nc.scalar.activation(sco_f[:], att_grp[:], AF.Exp,
                                 bias=nmax[:], accum_out=sume[:])
            nc.vector.reciprocal(rsum[:], sume[:])
            nc.vector.tensor_scalar_mul(sco_f[:], sco_f[:], rsum[:])
            nc.vector.tensor_copy(sco_b[0:GRP, :], sco_f[:])
            nc.sync.dma_start(score_o[g0:g0 + GRP, :], sco_f[:])

            scoT = [sct_p.tile([128, BL], BF16, tag="scoT", name=f"scoT{b}_{st}")
                    for st in range(STS)]
            for st in range(STS):
                nc.sync.dma_start(scoT[st][:],
                                  sco_b[0:BL, st * 128:(st + 1) * 128],
                                  transpose=True)
            for b2 in range(g0, g0 + GRP):
                wp = ps_w.tile([1, D], F32, tag="wps")
                for st in range(STS):
                    nc.tensor.matmul(wp[:], scoT[st][:, b2 - g0:b2 - g0 + 1],
                                     cn_sub(b2, st),
                                     start=(st == 0), stop=(st == STS - 1),
                                     skip_group_check=True)
                wrow = row_p.tile([1, D], BF16, tag="wrow", bufs=2)
                nc.vector.tensor_copy(wrow[:], wp[:])
                nc.sync.dma_start(wct_b[b2:b2 + 1, :], wrow[:])

    # ---- hidden = W_ctx @ wctx + b_ctx  -> hiddenT [h, b] ----
    wcT = [sct_p.tile([128, BL], BF16, tag="wcT", name=f"wcT{k}")
           for k in range(KD)]
    for k in range(KD):
        nc.sync.dma_start(wcT[k][:], wct_b[0:BL, k * 128:(k + 1) * 128],
                          transpose=True)
    for m in range(MH):
        hp = ps_sm.tile([128, BL], F32, tag="qps")
        for k in range(KD):
            nc.tensor.matmul(hp[:], Wc[:, k * H + m * 128: k * H + (m + 1) * 128],
                             wcT[k][:], start=(k == 0), stop=(k == KD - 1))
        hrow = row_p.tile([128, BL], F32, tag="hrow")
        nc.vector.tensor_scalar_add(hrow[:], hp[:], bia[:, MH + m: MH + m + 1])
        nc.sync.dma_start(hiddenT_o[m * 128:(m + 1) * 128, :], hrow[:])


_CACHE = {}


def _get_nc():
    if "nc" in _CACHE:
        return _CACHE["nc"]
    nc = bacc.Bacc("TRN2", target_bir_lowering=False, debug=False)
    t_ctx = nc.dram_tensor("contexts", [BL, S, D], F32, kind="ExternalInput").ap()
    t_inT = nc.dram_tensor("inputsT", [D, BL], BF16, kind="ExternalInput").ap()
    t_mad = nc.dram_tensor("maskadd", [BL, S], F32, kind="ExternalInput").ap()
    t_wi = nc.dram_tensor("W_inT", [D, H], BF16, kind="ExternalInput").ap()
    t_wc = nc.dram_tensor("W_ctxT", [D, H], BF16, kind="ExternalInput").ap()
    t_v = nc.dram_tensor("V", [128, MH], BF16, kind="ExternalInput").ap()
    t_bia = nc.dram_tensor("biases", [128, 2 * MH], F32, kind="ExternalInput").ap()
    t_hid = nc.dram_tensor("hiddenT", [H, BL], F32, kind="ExternalOutput").ap()
    t_sco = nc.dram_tensor("score", [BL, S], F32, kind="ExternalOutput").ap()
    with tile.TileContext(nc) as tc:
        with ExitStack() as es:
            _build_kernel(es, tc, (t_hid, t_sco),
                          (t_ctx, t_inT, t_mad, t_wi, t_wc, t_v, t_bia))
    nc.compile()
    _CACHE["nc"] = nc
    return nc


def _prep_in_maps(inputs, contexts, mask, W_in, b_in, W_ctx, b_ctx, V):
    W_inT = np.ascontiguousarray(W_in.T).astype(BF16_NP)
    W_ctxT = np.ascontiguousarray(W_ctx.T).astype(BF16_NP)
    V_t = np.ascontiguousarray(V.astype(BF16_NP).reshape(MH, 128).T)
    biases = np.ascontiguousarray(
        np.concatenate([(b_in + b_ctx).reshape(MH, 128).T,
                        b_ctx.reshape(MH, 128).T], axis=1)).astype(np.float32)
    maskadd = np.where(mask, np.float32(-1e30), np.float32(0.0)).astype(np.float32)

    in_maps = []
    for c in range(NCORES):
        rows = slice(c * BL, (c + 1) * BL)
        in_maps.append({
            "contexts": np.ascontiguousarray(contexts[rows]).astype(np.float32),
            "inputsT": np.ascontiguousarray(inputs[rows].T).astype(BF16_NP),
            "maskadd": np.ascontiguousarray(maskadd[rows]),
            "W_inT": W_inT,
            "W_ctxT": W_ctxT,
            "V": V_t,
            "biases": biases,
        })
    return in_maps


def kernel(inputs, contexts, mask, W_in, b_in, W_ctx, b_ctx, V, _trace=False):
    inputs = np.asarray(inputs)
    contexts = np.asarray(contexts)
    mask = np.asarray(mask)
    nc = _get_nc()
    in_maps = _prep_in_maps(inputs, contexts, mask,
                            np.asarray(W_in), np.asarray(b_in),
                            np.asarray(W_ctx), np.asarray(b_ctx), np.asarray(V))
    res = run_bass_kernel_spmd(nc, in_maps, core_ids=list(range(NCORES)),
                               trace=_trace)
    hidden = np.empty((B, H), np.float32)
    score = np.empty((B, S), np.float32)
    for c in range(NCORES):
        rows = slice(c * BL, (c + 1) * BL)
        hidden[rows] = res.results[c]["hiddenT"].T
        score[rows] = res.results[c]["score"]
    kernel.last_results = res
    return hidden, score
